# revision 1
# baseline (speedup 1.0000x reference)
"""Trainium2 Bass kernel for the NeuralODE (Tsit5, dense MLP vector field).

Strategy (data-parallel over batch, 8 cores, B=512 -> 64 rows/core):
  - All activations contracted on the tensor engine need the contraction
    dim on partitions ("feature-major"/FM). The state (y and the six
    Runge-Kutta slopes k_i) is kept FM as [64, 64] tiles.
  - Stage combinations arg_j = y + h*sum(a_ji k_i) are folded into the
    first MLP layer: z0_j = W0 y + b0 + sum_i a_ji * (W0 khat_i), where
    khat_i = h*(W2 h1_i + b2) absorbs h. The moving operands (a_ji W0^T)
    are host-precomputed constants, so the whole Tsit5 combination
    arithmetic runs inside matmul accumulation in PSUM.
  - Layer 1 (512x512) runs activation-stationary: lhsT = h0 FM chunks,
    rhs = W1^T chunks (N=512 moving), output batch-major in PSUM.
  - Batch-major hidden activations are re-transposed to FM with the PE
    transpose (4x [64,128] -> [128,64] per hidden).
  - The y update y += sum(B_i khat_i) is a PE matmul against constant
    (B_i * I) stationaries.

kernel(**inputs) takes FULL inputs, shards y0 across 8 cores host-side,
replicates the (host-preprocessed) weight constants, and gathers the
full [512, 16, 64] output.
"""

import numpy as np

# ---------------------------------------------------------------------------
# Tsit5 tableau (matches reference)
A21 = 0.161
A31, A32 = -0.008480655492356989, 0.335480655492357
A41, A42, A43 = 2.8971530571054935, -6.359448489975075, 4.3622954328695815
A51, A52, A53, A54 = 5.325864828439257, -11.748883564062828, 7.4955393428898365, -0.09249506636175525
A61, A62, A63, A64, A65 = 5.86145544294642, -12.92096931784711, 8.159367898576159, -0.071584973281401, -0.028269050394068383
B1, B2, B3, B4, B5, B6 = 0.09646076681806523, 0.01, 0.4798896504144996, 1.379008574103742, -3.290069515436081, 2.324710524099774

A_ROWS = {
    2: [A21],
    3: [A31, A32],
    4: [A41, A42, A43],
    5: [A51, A52, A53, A54],
    6: [A61, A62, A63, A64, A65],
}
B_W = [B1, B2, B3, B4, B5, B6]

B, D, W, T = 512, 64, 512, 16
SUBSTEPS = 4
NCORES = 8
BS = B // NCORES          # 64 batch rows per core
NINT = T - 1              # 15 intervals

USE_F32R = True           # relaxed fp32 matmuls (1 cyc/col at N>=512)
FULL_UNROLL = True

_CACHE = {}


def _patch_tile_drain():
    """This walrus build only accepts a single sync-wait on TPB_CTRL
    (Drain) instructions; TileContext's exit drain carries one wait per
    live proc. Spread them across single-wait drains."""
    import concourse.mybir as mybir
    from concourse.tile import TileContext
    from concourse.vector_clock import ScopedClock

    if getattr(TileContext, "_drain_patched", False):
        return

    def _patched(self, tick_clock, wait_clock):
        nc = self.nc
        drain_inst = nc.sync.drain()
        wait_clock.add_sem_waits(
            drain_inst.ins, ScopedClock({None: tick_clock.global_clock})
        )
        si = drain_inst.ins.sync_info
        if si is not None and len(si.on_wait) > 1:
            waits = list(si.on_wait)
            drain_inst.ins.sync_info = mybir.SyncInfo(
                on_wait=[waits[0]], on_update=list(si.on_update)
            )
            for wcond in waits[1:]:
                d2 = nc.sync.drain()
                d2.ins.sync_info = mybir.SyncInfo(on_wait=[wcond], on_update=[])
        nc.all_engine_barrier()
        assert self.sems is not None
        popped = nc._tile_sem_poison_stack.pop()
        assert popped is self._sem_poison
        nc.clear_and_free_semaphores(list(self.sems.allocated().values()))
        nc.all_engine_barrier()

    TileContext._drain_and_barrier = _patched
    TileContext._drain_patched = True

    # Walrus in this environment accepts only ONE sync-wait per lowered
    # instruction (setupSyncWait "Too many sync wait commands", seen on
    # Drain and on Matmult/S3_LW). Split every multi-wait instruction into
    # single-wait NoOps + the instruction at serialization time.
    import json as _json
    import concourse.bass as _bass

    if not getattr(_bass.Bass, "_mw_patched", False):
        _orig_to_json = _bass.Bass.to_json_bytes

        def _to_json_split(self, *a, **kw):
            raw = _orig_to_json(self, *a, **kw)
            m = _json.loads(raw)

            def fix_block(blk):
                insts = blk.get("instructions")
                if not isinstance(insts, list):
                    return
                out = []
                for ins in insts:
                    si = ins.get("sync_info")
                    if isinstance(si, dict):
                        w = si.get("on_wait") or []
                        if len(w) > 1:
                            for k, wc in enumerate(w[:-1]):
                                out.append({
                                    "debug": ins.get("debug", 0),
                                    "engine": ins["engine"],
                                    "ins": [], "outs": [],
                                    "name": f"{ins['name']}-mw{k}",
                                    "opcode": "NoOp",
                                    "sync_info": {"on_wait": [wc],
                                                  "on_update": []},
                                })
                            si["on_wait"] = [w[-1]]
                    out.append(ins)
                blk["instructions"] = out

            def rec(o):
                if isinstance(o, dict):
                    if "instructions" in o:
                        fix_block(o)
                    for v in o.values():
                        rec(v)
                elif isinstance(o, list):
                    for v in o:
                        rec(v)

            rec(m)
            return _json.dumps(m).encode()

        _bass.Bass.to_json_bytes = _to_json_split
        _bass.Bass._mw_patched = True


def _build_module(with_b1: bool, with_b2: bool):
    import concourse.bass as bass
    import concourse.mybir as mybir
    from concourse.tile import TileContext

    _patch_tile_drain()

    FT = mybir.dt.float32r if USE_F32R else mybir.dt.float32
    F32 = mybir.dt.float32
    AFT = mybir.ActivationFunctionType

    nc = bass.Bass()

    # ---- DRAM I/O ----
    T0I_d = nc.dram_tensor("T0I", [128, BS], FT, kind="ExternalInput")
    MW0_d = nc.dram_tensor("MW0", [128, W], FT, kind="ExternalInput")
    MWK_d = nc.dram_tensor("MWK", [D, 15, W], FT, kind="ExternalInput")
    W1T_d = nc.dram_tensor("W1T", [128, 4, W], FT, kind="ExternalInput")
    W2TH_d = nc.dram_tensor("W2TH", [128, NINT, 4, D], FT, kind="ExternalInput")
    if with_b2:
        HB2_d = nc.dram_tensor("HB2", [1, NINT * D], FT, kind="ExternalInput")
    if with_b1:
        B1R_d = nc.dram_tensor("B1R", [1, W], FT, kind="ExternalInput")
    if with_b1 or with_b2:
        ONESR_d = nc.dram_tensor("ONESR", [1, BS], FT, kind="ExternalInput")
    UY_d = nc.dram_tensor("UY", [128, D], FT, kind="ExternalInput")
    UK_d = nc.dram_tensor("UK", [D, 6 * D], FT, kind="ExternalInput")
    IDT_d = nc.dram_tensor("IDT", [D, D], FT, kind="ExternalInput")
    YS = nc.dram_tensor("YS", [NINT, D, BS], FT, kind="ExternalOutput")

    with TileContext(nc) as tc:
        with (
            tc.tile_pool(name="const", bufs=1) as cpool,
            tc.tile_pool(name="state", bufs=1) as stpool,
            tc.tile_pool(name="work", bufs=3) as wpool,
            tc.tile_pool(name="zp", bufs=2, space="PSUM") as zpool,
            tc.tile_pool(name="hTp", bufs=2, space="PSUM") as hTpool,
            tc.tile_pool(name="kyp", bufs=2, space="PSUM") as kypool,
        ):
            # ---- constants -> SBUF ----
            MW0 = cpool.tile([128, W], FT, tag="MW0")
            nc.sync.dma_start(MW0[:], MW0_d[:, :])
            MWK = cpool.tile([D, 15 * W], FT, tag="MWK")
            nc.sync.dma_start(MWK[:], MWK_d.rearrange("p k f -> p (k f)"))
            W1T = cpool.tile([128, 4 * W], FT, tag="W1T")
            nc.sync.dma_start(W1T[:], W1T_d.rearrange("p c f -> p (c f)"))
            W2TH = cpool.tile([128, NINT * 4 * D], FT, tag="W2TH")
            nc.sync.dma_start(W2TH[:], W2TH_d.rearrange("p i c f -> p (i c f)"))
            if with_b2:
                HB2 = cpool.tile([1, NINT * D], FT, tag="HB2")
                nc.sync.dma_start(HB2[:], HB2_d[:, :])
            if with_b1:
                B1R = cpool.tile([1, W], FT, tag="B1R")
                nc.sync.dma_start(B1R[:], B1R_d[:, :])
            UY = cpool.tile([128, D], FT, tag="UY")
            nc.sync.dma_start(UY[:], UY_d[:, :])
            UK = cpool.tile([D, 6 * D], FT, tag="UK")
            nc.sync.dma_start(UK[:], UK_d[:, :])
            IDT = cpool.tile([D, D], FT, tag="IDT")
            nc.sync.dma_start(IDT[:], IDT_d[:, :])
            if with_b1 or with_b2:
                ONES = cpool.tile([1, BS], FT, tag="ONES")
                nc.sync.dma_start(ONES[:], ONESR_d[:, :])

            # ---- state ----
            # T0: rows 0:64 = y (FM), rows 64:126 = 0, row 127 = ones
            # (host-initialized in one DMA)
            T0 = stpool.tile([128, BS], FT, tag="T0")
            nc.sync.dma_start(T0[:], T0I_d[:, :])
            K = [
                stpool.tile([D, BS], FT, tag=f"K{i}", name=f"K{i}")
                for i in range(6)
            ]

            mwk_idx = {}
            n = 0
            for j in range(2, 7):
                for i2 in range(len(A_ROWS[j])):
                    mwk_idx[(j, i2)] = n
                    n += 1

            def substep(i):
                for j in range(1, 7):
                    # ---- L0 (+ folded Tsit5 combination) -> z0 [64b, 512] BM
                    z0 = zpool.tile([BS, W], F32, tag="z")
                    terms = [(T0[:, :], MW0[:, :])]
                    for i2 in range(j - 1):
                        m = mwk_idx[(j, i2)]
                        terms.append((K[i2][:, :], MWK[:, m * W:(m + 1) * W]))
                    for c, (lhsT, rhs) in enumerate(terms):
                        nc.tensor.matmul(
                            z0[:], lhsT, rhs,
                            start=(c == 0), stop=(c == len(terms) - 1),
                        )
                    # ---- softplus -> h0 BM:
                    # r = relu(z-44); c = z-r (= min(z,44));
                    # out = ln(1+exp(c)) + r   (exact in fp32)
                    r0 = wpool.tile([BS, W], FT, tag="rp")
                    nc.vector.tensor_scalar(
                        r0[:], z0[:], 44.0, 0.0,
                        op0=mybir.AluOpType.subtract, op1=mybir.AluOpType.max,
                    )
                    c0 = wpool.tile([BS, W], FT, tag="cl")
                    nc.vector.tensor_sub(c0[:], z0[:], r0[:])
                    texp0 = wpool.tile([BS, W], FT, tag="texp")
                    nc.scalar.activation(texp0[:], c0[:], AFT.Exp)
                    s0 = wpool.tile([BS, W], FT, tag="sp")
                    nc.scalar.activation(s0[:], texp0[:], AFT.Ln, bias=1.0)
                    h0 = wpool.tile([BS, W], FT, tag="h")
                    nc.vector.tensor_add(h0[:], s0[:], r0[:])
                    # ---- transpose h0 -> FM [512, 64] as [128, 4*64]
                    h0Tp = hTpool.tile([128, 4 * BS], FT, tag="hTp")
                    for c in range(4):
                        nc.tensor.transpose(
                            h0Tp[:, c * BS:(c + 1) * BS],
                            h0[:, c * 128:(c + 1) * 128],
                            IDT[:],
                        )
                    h0T = wpool.tile([128, 4 * BS], FT, tag="hT")
                    nc.scalar.copy(h0T[:], h0Tp[:])
                    # ---- L1 -> z1 [64b, 512] BM (b1 via ones-row rank-1 mm)
                    z1 = zpool.tile([BS, W], F32, tag="z")
                    for c in range(4):
                        nc.tensor.matmul(
                            z1[:],
                            h0T[:, c * BS:(c + 1) * BS],
                            W1T[:, c * W:(c + 1) * W],
                            start=(c == 0), stop=(c == 3 and not with_b1),
                        )
                    if with_b1:
                        nc.tensor.matmul(
                            z1[:], ONES[:, :], B1R[:, :],
                            start=False, stop=True,
                        )
                    # ---- softplus -> h1 BM (same 4-op form)
                    r1 = wpool.tile([BS, W], FT, tag="rp")
                    nc.vector.tensor_scalar(
                        r1[:], z1[:], 44.0, 0.0,
                        op0=mybir.AluOpType.subtract, op1=mybir.AluOpType.max,
                    )
                    c1 = wpool.tile([BS, W], FT, tag="cl")
                    nc.vector.tensor_sub(c1[:], z1[:], r1[:])
                    texp1 = wpool.tile([BS, W], FT, tag="texp")
                    nc.scalar.activation(texp1[:], c1[:], AFT.Exp)
                    s1 = wpool.tile([BS, W], FT, tag="sp")
                    nc.scalar.activation(s1[:], texp1[:], AFT.Ln, bias=1.0)
                    h1 = wpool.tile([BS, W], FT, tag="h")
                    nc.vector.tensor_add(h1[:], s1[:], r1[:])
                    # ---- transpose h1 -> FM
                    h1Tp = hTpool.tile([128, 4 * BS], FT, tag="hTp")
                    for c in range(4):
                        nc.tensor.transpose(
                            h1Tp[:, c * BS:(c + 1) * BS],
                            h1[:, c * 128:(c + 1) * 128],
                            IDT[:],
                        )
                    h1T = wpool.tile([128, 4 * BS], FT, tag="hT")
                    nc.scalar.copy(h1T[:], h1Tp[:])
                    # ---- L2: khat_j = h*(W2 h1 + b2), FM [64d, 64b]
                    kp = kypool.tile([D, BS], F32, tag="k")
                    for c in range(4):
                        nc.tensor.matmul(
                            kp[:],
                            W2TH[:, (i * 4 + c) * D:(i * 4 + c + 1) * D],
                            h1T[:, c * BS:(c + 1) * BS],
                            start=(c == 0), stop=(c == 3 and not with_b2),
                        )
                    if with_b2:
                        nc.tensor.matmul(
                            kp[:],
                            HB2[:, i * D:(i + 1) * D],
                            ONES[:, :],
                            start=False, stop=True,
                        )
                    nc.vector.tensor_copy(K[j - 1][:], kp[:])

                # ---- y update: y += sum B_i khat_i
                yn = kypool.tile([D, BS], F32, tag="k")
                nc.tensor.matmul(yn[:], UY[:, :], T0[:, :], start=True, stop=False)
                for i2 in range(6):
                    nc.tensor.matmul(
                        yn[:],
                        UK[:, i2 * D:(i2 + 1) * D],
                        K[i2][:, :],
                        start=False, stop=(i2 == 5),
                    )
                nc.vector.tensor_copy(T0[0:D, :], yn[:])

            for i in range(NINT):
                for _s in range(SUBSTEPS):
                    substep(i)
                nc.sync.dma_start(YS[i, :, :], T0[0:D, :])

    return nc


def _host_constants(ts, W0, b0, W1, b1, W2, b2):
    """Precompute all device constant tensors (fp32)."""
    f = np.float32
    ts = np.asarray(ts, f)
    W0, b0 = np.asarray(W0, f), np.asarray(b0, f)
    W1, b1 = np.asarray(W1, f), np.asarray(b1, f)
    W2, b2 = np.asarray(W2, f), np.asarray(b2, f)

    hs = (ts[1:] - ts[:-1]) / f(SUBSTEPS)          # [15]

    MW0 = np.zeros((128, W), f)
    MW0[0:D, :] = W0.T                              # y rows
    MW0[127, :] = b0                                # ones row -> +b0
    B1ROW = b1.reshape(1, W).copy()                 # [1, 512]

    MWK = np.zeros((D, 15, W), f)
    n = 0
    for j in range(2, 7):
        for a in A_ROWS[j]:
            MWK[:, n, :] = f(a) * W0.T
            n += 1

    W1T = np.zeros((128, 4, W), f)
    for c in range(4):
        W1T[:, c, :] = W1.T[c * 128:(c + 1) * 128, :]

    W2TH = np.zeros((128, NINT, 4, D), f)
    for i in range(NINT):
        for c in range(4):
            W2TH[:, i, c, :] = hs[i] * W2.T[c * 128:(c + 1) * 128, :]

    HB2 = np.zeros((1, NINT * D), f)
    for i in range(NINT):
        HB2[0, i * D:(i + 1) * D] = hs[i] * b2

    UY = np.zeros((128, D), f)
    UY[0:D, 0:D] = np.eye(D, dtype=f)

    UK = np.zeros((D, 6 * D), f)
    for i2 in range(6):
        UK[:, i2 * D:(i2 + 1) * D] = f(B_W[i2]) * np.eye(D, dtype=f)

    IDT = np.eye(D, dtype=f)

    return dict(MW0=MW0, MWK=MWK, W1T=W1T, W2TH=W2TH, HB2=HB2,
                UY=UY, UK=UK, IDT=IDT, B1ROW=B1ROW)


def kernel(ts, y0, W0, b0, W1, b1, W2, b2):
    from concourse.bass_utils import run_bass_kernel_spmd

    consts = _host_constants(ts, W0, b0, W1, b1, W2, b2)
    b1row = consts.pop("B1ROW")
    with_b1 = bool(np.any(b1row != 0))
    with_b2 = bool(np.any(consts["HB2"] != 0))
    if with_b1:
        consts["B1R"] = b1row
    if not with_b2:
        consts.pop("HB2")
    if with_b1 or with_b2:
        consts["ONESR"] = np.ones((1, BS), np.float32)

    key = ("nc", with_b1, with_b2)
    if key not in _CACHE:
        _CACHE[key] = _build_module(with_b1, with_b2)
    nc = _CACHE[key]

    y0 = np.asarray(y0, np.float32)
    in_maps = []
    for c in range(NCORES):
        shard = y0[c * BS:(c + 1) * BS, :]          # [64, 64]
        t0i = np.zeros((128, BS), np.float32)
        t0i[0:D, :] = shard.T
        t0i[127, :] = 1.0
        m = {"T0I": t0i}
        m.update({k: v for k, v in consts.items()})
        in_maps.append(m)

    res = run_bass_kernel_spmd(nc, in_maps, list(range(NCORES)))

    out = np.zeros((B, T, D), np.float32)
    out[:, 0, :] = y0
    for c in range(NCORES):
        ys = res.results[c]["YS"]                   # [15, 64d, 64b]
        out[c * BS:(c + 1) * BS, 1:, :] = ys.transpose(2, 0, 1)
    return out



# revision 2
# speedup vs baseline: 102.3056x; 102.3056x over previous
"""Trainium2 Bass kernel for the NeuralODE (Tsit5, dense MLP vector field).

Strategy (data-parallel over batch, 8 cores, B=512 -> 64 rows/core):
  - All activations contracted on the tensor engine need the contraction
    dim on partitions ("feature-major"/FM). The state (y and the six
    Runge-Kutta slopes k_i) is kept FM as [64, 64] tiles.
  - Stage combinations arg_j = y + h*sum(a_ji k_i) are folded into the
    first MLP layer: z0_j = W0 y + b0 + sum_i a_ji * (W0 khat_i), where
    khat_i = h*(W2 h1_i + b2) absorbs h. The moving operands (a_ji W0^T)
    are host-precomputed constants, so the whole Tsit5 combination
    arithmetic runs inside matmul accumulation in PSUM.
  - Layer 1 (512x512) runs activation-stationary: lhsT = h0 FM chunks,
    rhs = W1^T chunks (N=512 moving), output batch-major in PSUM.
  - Batch-major hidden activations are re-transposed to FM with the PE
    transpose (4x [64,128] -> [128,64] per hidden).
  - The y update y += sum(B_i khat_i) is a PE matmul against constant
    (B_i * I) stationaries.

kernel(**inputs) takes FULL inputs, shards y0 across 8 cores host-side,
replicates the (host-preprocessed) weight constants, and gathers the
full [512, 16, 64] output.
"""

import numpy as np

# ---------------------------------------------------------------------------
# Tsit5 tableau (matches reference)
A21 = 0.161
A31, A32 = -0.008480655492356989, 0.335480655492357
A41, A42, A43 = 2.8971530571054935, -6.359448489975075, 4.3622954328695815
A51, A52, A53, A54 = 5.325864828439257, -11.748883564062828, 7.4955393428898365, -0.09249506636175525
A61, A62, A63, A64, A65 = 5.86145544294642, -12.92096931784711, 8.159367898576159, -0.071584973281401, -0.028269050394068383
B1, B2, B3, B4, B5, B6 = 0.09646076681806523, 0.01, 0.4798896504144996, 1.379008574103742, -3.290069515436081, 2.324710524099774

A_ROWS = {
    2: [A21],
    3: [A31, A32],
    4: [A41, A42, A43],
    5: [A51, A52, A53, A54],
    6: [A61, A62, A63, A64, A65],
}
B_W = [B1, B2, B3, B4, B5, B6]

B, D, W, T = 512, 64, 512, 16
SUBSTEPS = 4
NCORES = 8
BS = B // NCORES          # 64 batch rows per core
NINT = T - 1              # 15 intervals

USE_F32R = True           # relaxed fp32 matmuls (1 cyc/col at N>=512)
FULL_UNROLL = True

_CACHE = {}


def _patch_tile_drain():
    """This walrus build only accepts a single sync-wait on TPB_CTRL
    (Drain) instructions; TileContext's exit drain carries one wait per
    live proc. Spread them across single-wait drains."""
    import concourse.mybir as mybir
    from concourse.tile import TileContext
    from concourse.vector_clock import ScopedClock

    if getattr(TileContext, "_drain_patched", False):
        return

    def _patched(self, tick_clock, wait_clock):
        nc = self.nc
        drain_inst = nc.sync.drain()
        wait_clock.add_sem_waits(
            drain_inst.ins, ScopedClock({None: tick_clock.global_clock})
        )
        si = drain_inst.ins.sync_info
        if si is not None and len(si.on_wait) > 1:
            waits = list(si.on_wait)
            drain_inst.ins.sync_info = mybir.SyncInfo(
                on_wait=[waits[0]], on_update=list(si.on_update)
            )
            for wcond in waits[1:]:
                d2 = nc.sync.drain()
                d2.ins.sync_info = mybir.SyncInfo(on_wait=[wcond], on_update=[])
        nc.all_engine_barrier()
        assert self.sems is not None
        popped = nc._tile_sem_poison_stack.pop()
        assert popped is self._sem_poison
        nc.clear_and_free_semaphores(list(self.sems.allocated().values()))
        nc.all_engine_barrier()

    TileContext._drain_and_barrier = _patched
    TileContext._drain_patched = True

    # Walrus in this environment accepts only ONE sync-wait per lowered
    # instruction (setupSyncWait "Too many sync wait commands", seen on
    # Drain and on Matmult/S3_LW). Split every multi-wait instruction into
    # single-wait NoOps + the instruction at serialization time.
    import json as _json
    import concourse.bass as _bass

    if not getattr(_bass.Bass, "_mw_patched", False):
        _orig_to_json = _bass.Bass.to_json_bytes

        def _to_json_split(self, *a, **kw):
            raw = _orig_to_json(self, *a, **kw)
            m = _json.loads(raw)

            def fix_block(blk):
                insts = blk.get("instructions")
                if not isinstance(insts, list):
                    return
                out = []
                for ins in insts:
                    si = ins.get("sync_info")
                    if isinstance(si, dict):
                        w = si.get("on_wait") or []
                        if len(w) > 1:
                            for k, wc in enumerate(w[:-1]):
                                out.append({
                                    "debug": ins.get("debug", 0),
                                    "engine": ins["engine"],
                                    "ins": [], "outs": [],
                                    "name": f"{ins['name']}-mw{k}",
                                    "opcode": "NoOp",
                                    "sync_info": {"on_wait": [wc],
                                                  "on_update": []},
                                })
                            si["on_wait"] = [w[-1]]
                    out.append(ins)
                blk["instructions"] = out

            def rec(o):
                if isinstance(o, dict):
                    if "instructions" in o:
                        fix_block(o)
                    for v in o.values():
                        rec(v)
                elif isinstance(o, list):
                    for v in o:
                        rec(v)

            rec(m)
            return _json.dumps(m).encode()

        _bass.Bass.to_json_bytes = _to_json_split
        _bass.Bass._mw_patched = True


def _build_module(with_b1: bool, with_b2: bool):
    import concourse.bass as bass
    import concourse.mybir as mybir
    from concourse.tile import TileContext

    _patch_tile_drain()

    FT = mybir.dt.float32r if USE_F32R else mybir.dt.float32
    F32 = mybir.dt.float32
    AFT = mybir.ActivationFunctionType

    nc = bass.Bass()

    # ---- DRAM I/O ----
    T0I_d = nc.dram_tensor("T0I", [128, BS], FT, kind="ExternalInput")
    MW0_d = nc.dram_tensor("MW0", [128, W], FT, kind="ExternalInput")
    MWK_d = nc.dram_tensor("MWK", [D, 15, W], FT, kind="ExternalInput")
    W1T_d = nc.dram_tensor("W1T", [128, 4, W], FT, kind="ExternalInput")
    W2TH_d = nc.dram_tensor("W2TH", [128, NINT, 4, D], FT, kind="ExternalInput")
    if with_b2:
        HB2_d = nc.dram_tensor("HB2", [1, NINT * D], FT, kind="ExternalInput")
    if with_b1:
        B1R_d = nc.dram_tensor("B1R", [1, W], FT, kind="ExternalInput")
    if with_b1 or with_b2:
        ONESR_d = nc.dram_tensor("ONESR", [1, BS], FT, kind="ExternalInput")
    UY_d = nc.dram_tensor("UY", [128, D], FT, kind="ExternalInput")
    UK_d = nc.dram_tensor("UK", [D, 6 * D], FT, kind="ExternalInput")
    IDT_d = nc.dram_tensor("IDT", [D, D], FT, kind="ExternalInput")
    YS = nc.dram_tensor("YS", [NINT, D, BS], FT, kind="ExternalOutput")

    with TileContext(nc) as tc:
        with (
            tc.tile_pool(name="const", bufs=1) as cpool,
            tc.tile_pool(name="state", bufs=1) as stpool,
            tc.tile_pool(name="work", bufs=3) as wpool,
            tc.tile_pool(name="zp", bufs=2, space="PSUM") as zpool,
            tc.tile_pool(name="hTp", bufs=2, space="PSUM") as hTpool,
            tc.tile_pool(name="kyp", bufs=2, space="PSUM") as kypool,
        ):
            # ---- constants -> SBUF ----
            MW0 = cpool.tile([128, W], FT, tag="MW0")
            nc.sync.dma_start(MW0[:], MW0_d[:, :])
            MWK = cpool.tile([D, 15 * W], FT, tag="MWK")
            nc.sync.dma_start(MWK[:], MWK_d.rearrange("p k f -> p (k f)"))
            W1T = cpool.tile([128, 4 * W], FT, tag="W1T")
            nc.sync.dma_start(W1T[:], W1T_d.rearrange("p c f -> p (c f)"))
            W2TH = cpool.tile([128, NINT * 4 * D], FT, tag="W2TH")
            nc.sync.dma_start(W2TH[:], W2TH_d.rearrange("p i c f -> p (i c f)"))
            if with_b2:
                HB2 = cpool.tile([1, NINT * D], FT, tag="HB2")
                nc.sync.dma_start(HB2[:], HB2_d[:, :])
            if with_b1:
                B1R = cpool.tile([1, W], FT, tag="B1R")
                nc.sync.dma_start(B1R[:], B1R_d[:, :])
            UY = cpool.tile([128, D], FT, tag="UY")
            nc.sync.dma_start(UY[:], UY_d[:, :])
            UK = cpool.tile([D, 6 * D], FT, tag="UK")
            nc.sync.dma_start(UK[:], UK_d[:, :])
            IDT = cpool.tile([D, D], FT, tag="IDT")
            nc.sync.dma_start(IDT[:], IDT_d[:, :])
            if with_b1 or with_b2:
                ONES = cpool.tile([1, BS], FT, tag="ONES")
                nc.sync.dma_start(ONES[:], ONESR_d[:, :])

            # ---- state ----
            # T0: rows 0:64 = y (FM), rows 64:126 = 0, row 127 = ones
            # (host-initialized in one DMA)
            T0 = stpool.tile([128, BS], FT, tag="T0")
            nc.sync.dma_start(T0[:], T0I_d[:, :])
            K = [
                stpool.tile([D, BS], FT, tag=f"K{i}", name=f"K{i}")
                for i in range(6)
            ]

            mwk_idx = {}
            n = 0
            for j in range(2, 7):
                for i2 in range(len(A_ROWS[j])):
                    mwk_idx[(j, i2)] = n
                    n += 1

            def substep(i):
                for j in range(1, 7):
                    # ---- L0 (+ folded Tsit5 combination) -> z0 [64b, 512] BM
                    z0 = zpool.tile([BS, W], F32, tag="z")
                    terms = [(T0[:, :], MW0[:, :])]
                    for i2 in range(j - 1):
                        m = mwk_idx[(j, i2)]
                        terms.append((K[i2][:, :], MWK[:, m * W:(m + 1) * W]))
                    for c, (lhsT, rhs) in enumerate(terms):
                        nc.tensor.matmul(
                            z0[:], lhsT, rhs,
                            start=(c == 0), stop=(c == len(terms) - 1),
                        )
                    # ---- softplus -> h0 BM:
                    # r = relu(z-44); c = z-r (= min(z,44));
                    # out = ln(1+exp(c)) + r   (exact in fp32)
                    r0 = wpool.tile([BS, W], FT, tag="rp")
                    nc.vector.tensor_scalar(
                        r0[:], z0[:], 44.0, 0.0,
                        op0=mybir.AluOpType.subtract, op1=mybir.AluOpType.max,
                    )
                    c0 = wpool.tile([BS, W], FT, tag="cl")
                    nc.vector.tensor_sub(c0[:], z0[:], r0[:])
                    texp0 = wpool.tile([BS, W], FT, tag="texp")
                    nc.scalar.activation(texp0[:], c0[:], AFT.Exp)
                    s0 = wpool.tile([BS, W], FT, tag="sp")
                    nc.scalar.activation(s0[:], texp0[:], AFT.Ln, bias=1.0)
                    h0 = wpool.tile([BS, W], FT, tag="h")
                    nc.vector.tensor_add(h0[:], s0[:], r0[:])
                    # ---- transpose h0 -> FM [512, 64] as [128, 4*64]
                    h0Tp = hTpool.tile([128, 4 * BS], FT, tag="hTp")
                    for c in range(4):
                        nc.tensor.transpose(
                            h0Tp[:, c * BS:(c + 1) * BS],
                            h0[:, c * 128:(c + 1) * 128],
                            IDT[:],
                        )
                    h0T = wpool.tile([128, 4 * BS], FT, tag="hT")
                    nc.scalar.copy(h0T[:], h0Tp[:])
                    # ---- L1 -> z1 [64b, 512] BM (b1 via ones-row rank-1 mm)
                    z1 = zpool.tile([BS, W], F32, tag="z")
                    for c in range(4):
                        nc.tensor.matmul(
                            z1[:],
                            h0T[:, c * BS:(c + 1) * BS],
                            W1T[:, c * W:(c + 1) * W],
                            start=(c == 0), stop=(c == 3 and not with_b1),
                        )
                    if with_b1:
                        nc.tensor.matmul(
                            z1[:], ONES[:, :], B1R[:, :],
                            start=False, stop=True,
                        )
                    # ---- softplus -> h1 BM (same 4-op form)
                    r1 = wpool.tile([BS, W], FT, tag="rp")
                    nc.vector.tensor_scalar(
                        r1[:], z1[:], 44.0, 0.0,
                        op0=mybir.AluOpType.subtract, op1=mybir.AluOpType.max,
                    )
                    c1 = wpool.tile([BS, W], FT, tag="cl")
                    nc.vector.tensor_sub(c1[:], z1[:], r1[:])
                    texp1 = wpool.tile([BS, W], FT, tag="texp")
                    nc.scalar.activation(texp1[:], c1[:], AFT.Exp)
                    s1 = wpool.tile([BS, W], FT, tag="sp")
                    nc.scalar.activation(s1[:], texp1[:], AFT.Ln, bias=1.0)
                    h1 = wpool.tile([BS, W], FT, tag="h")
                    nc.vector.tensor_add(h1[:], s1[:], r1[:])
                    # ---- transpose h1 -> FM
                    h1Tp = hTpool.tile([128, 4 * BS], FT, tag="hTp")
                    for c in range(4):
                        nc.tensor.transpose(
                            h1Tp[:, c * BS:(c + 1) * BS],
                            h1[:, c * 128:(c + 1) * 128],
                            IDT[:],
                        )
                    h1T = wpool.tile([128, 4 * BS], FT, tag="hT")
                    nc.scalar.copy(h1T[:], h1Tp[:])
                    # ---- L2: khat_j = h*(W2 h1 + b2), FM [64d, 64b]
                    kp = kypool.tile([D, BS], F32, tag="k")
                    for c in range(4):
                        nc.tensor.matmul(
                            kp[:],
                            W2TH[:, (i * 4 + c) * D:(i * 4 + c + 1) * D],
                            h1T[:, c * BS:(c + 1) * BS],
                            start=(c == 0), stop=(c == 3 and not with_b2),
                        )
                    if with_b2:
                        nc.tensor.matmul(
                            kp[:],
                            HB2[:, i * D:(i + 1) * D],
                            ONES[:, :],
                            start=False, stop=True,
                        )
                    nc.vector.tensor_copy(K[j - 1][:], kp[:])

                # ---- y update: y += sum B_i khat_i
                yn = kypool.tile([D, BS], F32, tag="k")
                nc.tensor.matmul(yn[:], UY[:, :], T0[:, :], start=True, stop=False)
                for i2 in range(6):
                    nc.tensor.matmul(
                        yn[:],
                        UK[:, i2 * D:(i2 + 1) * D],
                        K[i2][:, :],
                        start=False, stop=(i2 == 5),
                    )
                nc.vector.tensor_copy(T0[0:D, :], yn[:])

            for i in range(NINT):
                for _s in range(SUBSTEPS):
                    substep(i)
                nc.sync.dma_start(YS[i, :, :], T0[0:D, :])

    return nc


def _host_constants(ts, W0, b0, W1, b1, W2, b2):
    """Precompute all device constant tensors (fp32)."""
    f = np.float32
    ts = np.asarray(ts, f)
    W0, b0 = np.asarray(W0, f), np.asarray(b0, f)
    W1, b1 = np.asarray(W1, f), np.asarray(b1, f)
    W2, b2 = np.asarray(W2, f), np.asarray(b2, f)

    hs = (ts[1:] - ts[:-1]) / f(SUBSTEPS)          # [15]

    MW0 = np.zeros((128, W), f)
    MW0[0:D, :] = W0.T                              # y rows
    MW0[127, :] = b0                                # ones row -> +b0
    B1ROW = b1.reshape(1, W).copy()                 # [1, 512]

    MWK = np.zeros((D, 15, W), f)
    n = 0
    for j in range(2, 7):
        for a in A_ROWS[j]:
            MWK[:, n, :] = f(a) * W0.T
            n += 1

    W1T = np.zeros((128, 4, W), f)
    for c in range(4):
        W1T[:, c, :] = W1.T[c * 128:(c + 1) * 128, :]

    W2TH = np.zeros((128, NINT, 4, D), f)
    for i in range(NINT):
        for c in range(4):
            W2TH[:, i, c, :] = hs[i] * W2.T[c * 128:(c + 1) * 128, :]

    HB2 = np.zeros((1, NINT * D), f)
    for i in range(NINT):
        HB2[0, i * D:(i + 1) * D] = hs[i] * b2

    UY = np.zeros((128, D), f)
    UY[0:D, 0:D] = np.eye(D, dtype=f)

    UK = np.zeros((D, 6 * D), f)
    for i2 in range(6):
        UK[:, i2 * D:(i2 + 1) * D] = f(B_W[i2]) * np.eye(D, dtype=f)

    IDT = np.eye(D, dtype=f)

    return dict(MW0=MW0, MWK=MWK, W1T=W1T, W2TH=W2TH, HB2=HB2,
                UY=UY, UK=UK, IDT=IDT, B1ROW=B1ROW)


def _split_consts(ts, W0, b0, W1, b1, W2, b2):
    consts = _host_constants(ts, W0, b0, W1, b1, W2, b2)
    b1row = consts.pop("B1ROW")
    with_b1 = bool(np.any(b1row != 0))
    with_b2 = bool(np.any(consts["HB2"] != 0))
    if with_b1:
        consts["B1R"] = b1row
    if not with_b2:
        consts.pop("HB2")
    if with_b1 or with_b2:
        consts["ONESR"] = np.ones((1, BS), np.float32)
    return consts, with_b1, with_b2


def _t0i_concat(y0):
    """Global [NCORES*128, BS] T0 initializer (per-core shards stacked)."""
    t0i = np.zeros((NCORES, 128, BS), np.float32)
    for c in range(NCORES):
        t0i[c, 0:D, :] = y0[c * BS:(c + 1) * BS, :].T
        t0i[c, 127, :] = 1.0
    return t0i.reshape(NCORES * 128, BS)


class _Runtime:
    """Cached compiled callable + device-resident constant inputs.

    run_bass_kernel_spmd rebuilds a fresh jax.jit closure (full XLA+NEFF
    recompile) and re-transfers every replicated constant on EVERY call.
    This caches both: the jit function is built once, constants are
    device_put once with the mesh sharding, and warm calls only upload the
    small y0-derived state tile and download YS.
    """

    def __init__(self, nc, consts):
        import jax
        import jax.numpy as jnp
        from jax.sharding import Mesh, PartitionSpec, NamedSharding
        try:
            from jax.experimental.shard_map import shard_map
        except ImportError:
            from jax import shard_map
        import concourse.mybir as mybir
        from concourse import bass2jax as b2j

        b2j.install_neuronx_cc_hook()

        self.jnp = jnp
        self.np = np

        in_names, out_names, out_avals = [], [], []
        partition_name = (
            nc.partition_id_tensor.name if nc.partition_id_tensor else None
        )
        for alloc in nc.m.functions[0].allocations:
            if not isinstance(alloc, mybir.MemoryLocationSet):
                continue
            name = alloc.memorylocations[0].name
            if alloc.kind == "ExternalInput":
                if name != partition_name:
                    in_names.append(name)
            elif alloc.kind == "ExternalOutput":
                out_names.append(name)
                out_avals.append(jax.core.ShapedArray(
                    tuple(alloc.tensor_shape), mybir.dt.np(alloc.dtype)))

        if nc.dbg_addr is not None and nc.dbg_callbacks:
            raise RuntimeError("dbg_callbacks unsupported in cached path")

        self.in_names = list(in_names)
        self.out_names = list(out_names)
        self.out_avals = list(out_avals)

        devices = jax.devices()[:NCORES]
        assert len(devices) == NCORES
        mesh = Mesh(np.asarray(devices), ("core",))
        P = PartitionSpec
        self.sharding = NamedSharding(mesh, P("core"))

        bind_in_names = list(in_names)
        if nc.dbg_addr is not None:
            bind_in_names.append(nc.dbg_addr.name)
        if partition_name is not None:
            bind_in_names.append(partition_name)
        n_args = len(bind_in_names) - (1 if partition_name is not None else 0)
        has_dbg = nc.dbg_addr is not None

        def _body(*args):
            operands = list(args)
            if partition_name is not None:
                operands.append(b2j.partition_id_tensor())
            outs = b2j._bass_exec_p.bind(
                *operands,
                out_avals=tuple(out_avals),
                in_names=tuple(bind_in_names),
                out_names=tuple(out_names),
                lowering_input_output_aliases=(),
                sim_require_finite=True,
                sim_require_nnan=True,
                nc=nc,
            )
            return tuple(outs)

        self.fn = jax.jit(
            shard_map(
                _body, mesh=mesh,
                in_specs=(P("core"),) * n_args,
                out_specs=(P("core"),) * len(out_names),
                check_rep=False,
            ),
            keep_unused=True,
        )
        self.has_dbg = has_dbg

        # device-resident replicated constants (concat over cores, sharded)
        self.const_dev = {}
        for k, v in consts.items():
            g = np.concatenate([v] * NCORES, axis=0)
            self.const_dev[k] = jax.device_put(g, self.sharding)
        if has_dbg:
            self.const_dev["__dbg"] = jax.device_put(
                np.zeros((NCORES * 1, 2), np.uint32), self.sharding)

    def run(self, t0i_global):
        import jax
        args = []
        for name in self.in_names:
            if name == "T0I":
                args.append(jax.device_put(t0i_global, self.sharding))
            else:
                args.append(self.const_dev[name])
        if self.has_dbg:
            args.append(self.const_dev["__dbg"])
        outs = self.fn(*args)
        return {name: np.asarray(outs[i])
                for i, name in enumerate(self.out_names)}


def _kernel_fallback(consts, with_b1, with_b2, y0):
    from concourse.bass_utils import run_bass_kernel_spmd

    key = ("nc", with_b1, with_b2)
    if key not in _CACHE:
        _CACHE[key] = _build_module(with_b1, with_b2)
    nc = _CACHE[key]

    in_maps = []
    for c in range(NCORES):
        shard = y0[c * BS:(c + 1) * BS, :]          # [64, 64]
        t0i = np.zeros((128, BS), np.float32)
        t0i[0:D, :] = shard.T
        t0i[127, :] = 1.0
        m = {"T0I": t0i}
        m.update({k: v for k, v in consts.items()})
        in_maps.append(m)

    res = run_bass_kernel_spmd(nc, in_maps, list(range(NCORES)))
    return [res.results[c]["YS"] for c in range(NCORES)]


def kernel(ts, y0, W0, b0, W1, b1, W2, b2):
    import sys

    consts, with_b1, with_b2 = _split_consts(ts, W0, b0, W1, b1, W2, b2)
    y0 = np.asarray(y0, np.float32)

    ys_per_core = None
    try:
        rt_key = ("rt", with_b1, with_b2)
        rt = _CACHE.get(rt_key)
        if rt is not None and not all(
            np.array_equal(rt[1][k], consts[k]) for k in consts
        ):
            rt = None                                # weights changed
        if rt is None:
            nc_key = ("nc", with_b1, with_b2)
            if nc_key not in _CACHE:
                _CACHE[nc_key] = _build_module(with_b1, with_b2)
            rt = (_Runtime(_CACHE[nc_key], consts), consts)
            _CACHE[rt_key] = rt
        res = rt[0].run(_t0i_concat(y0))
        ys = res["YS"].reshape(NCORES, NINT, D, BS)
        ys_per_core = [ys[c] for c in range(NCORES)]
    except Exception as e:                           # pragma: no cover
        print(f"kernel: cached path failed ({e!r}); falling back",
              file=sys.stderr)
        ys_per_core = _kernel_fallback(consts, with_b1, with_b2, y0)

    out = np.zeros((B, T, D), np.float32)
    out[:, 0, :] = y0
    for c in range(NCORES):
        out[c * BS:(c + 1) * BS, 1:, :] = ys_per_core[c].transpose(2, 0, 1)
    return out



# revision 17
# speedup vs baseline: 156.0835x; 1.5257x over previous
"""Trainium2 Bass kernel for the NeuralODE (Tsit5, dense MLP vector field).

Strategy (data-parallel over batch, 8 cores, B=512 -> 64 rows/core):
  - All activations contracted on the tensor engine need the contraction
    dim on partitions ("feature-major"/FM). The state (y and the six
    Runge-Kutta slopes k_i) is kept FM as [64, 64] tiles.
  - Stage combinations arg_j = y + h*sum(a_ji k_i) are folded into the
    first MLP layer: z0_j = W0 y + b0 + sum_i a_ji * (W0 khat_i), where
    khat_i = h*(W2 h1_i + b2) absorbs h. The moving operands (a_ji W0^T)
    are host-precomputed constants, so the whole Tsit5 combination
    arithmetic runs inside matmul accumulation in PSUM.
  - Layer 1 (512x512) runs activation-stationary: lhsT = h0 FM chunks,
    rhs = W1^T chunks (N=512 moving), output batch-major in PSUM.
  - Batch-major hidden activations are re-transposed to FM with the PE
    transpose (4x [64,128] -> [128,64] per hidden).
  - The y update y += sum(B_i khat_i) is a PE matmul against constant
    (B_i * I) stationaries.

kernel(**inputs) takes FULL inputs, shards y0 across 8 cores host-side,
replicates the (host-preprocessed) weight constants, and gathers the
full [512, 16, 64] output.
"""

import numpy as np

# ---------------------------------------------------------------------------
# Tsit5 tableau (matches reference)
A21 = 0.161
A31, A32 = -0.008480655492356989, 0.335480655492357
A41, A42, A43 = 2.8971530571054935, -6.359448489975075, 4.3622954328695815
A51, A52, A53, A54 = 5.325864828439257, -11.748883564062828, 7.4955393428898365, -0.09249506636175525
A61, A62, A63, A64, A65 = 5.86145544294642, -12.92096931784711, 8.159367898576159, -0.071584973281401, -0.028269050394068383
B1, B2, B3, B4, B5, B6 = 0.09646076681806523, 0.01, 0.4798896504144996, 1.379008574103742, -3.290069515436081, 2.324710524099774

A_ROWS = {
    2: [A21],
    3: [A31, A32],
    4: [A41, A42, A43],
    5: [A51, A52, A53, A54],
    6: [A61, A62, A63, A64, A65],
}
B_W = [B1, B2, B3, B4, B5, B6]

B, D, W, T = 512, 64, 512, 16
SUBSTEPS = 4
NCORES = 8
BS = B // NCORES          # 64 batch rows per core
NINT = T - 1              # 15 intervals

USE_F32R = True           # relaxed fp32 matmuls (1 cyc/col at N>=512)
FULL_UNROLL = True

_CACHE = {}


def _patch_tile_drain():
    """This walrus build only accepts a single sync-wait on TPB_CTRL
    (Drain) instructions; TileContext's exit drain carries one wait per
    live proc. Spread them across single-wait drains."""
    import concourse.mybir as mybir
    from concourse.tile import TileContext
    from concourse.vector_clock import ScopedClock

    if getattr(TileContext, "_drain_patched", False):
        return

    def _patched(self, tick_clock, wait_clock):
        nc = self.nc
        drain_inst = nc.sync.drain()
        wait_clock.add_sem_waits(
            drain_inst.ins, ScopedClock({None: tick_clock.global_clock})
        )
        si = drain_inst.ins.sync_info
        if si is not None and len(si.on_wait) > 1:
            waits = list(si.on_wait)
            drain_inst.ins.sync_info = mybir.SyncInfo(
                on_wait=[waits[0]], on_update=list(si.on_update)
            )
            for wcond in waits[1:]:
                d2 = nc.sync.drain()
                d2.ins.sync_info = mybir.SyncInfo(on_wait=[wcond], on_update=[])
        nc.all_engine_barrier()
        assert self.sems is not None
        popped = nc._tile_sem_poison_stack.pop()
        assert popped is self._sem_poison
        nc.clear_and_free_semaphores(list(self.sems.allocated().values()))
        nc.all_engine_barrier()

    TileContext._drain_and_barrier = _patched
    TileContext._drain_patched = True

    # Walrus in this environment accepts only ONE sync-wait per lowered
    # instruction (setupSyncWait "Too many sync wait commands", seen on
    # Drain and on Matmult/S3_LW). Split every multi-wait instruction into
    # single-wait NoOps + the instruction at serialization time.
    import json as _json
    import concourse.bass as _bass

    if not getattr(_bass.Bass, "_mw_patched", False):
        _orig_to_json = _bass.Bass.to_json_bytes

        def _to_json_split(self, *a, **kw):
            raw = _orig_to_json(self, *a, **kw)
            m = _json.loads(raw)

            def fix_block(blk):
                insts = blk.get("instructions")
                if not isinstance(insts, list):
                    return
                out = []
                for ins in insts:
                    si = ins.get("sync_info")
                    if isinstance(si, dict):
                        w = si.get("on_wait") or []
                        if len(w) > 1:
                            for k, wc in enumerate(w[:-1]):
                                out.append({
                                    "debug": ins.get("debug", 0),
                                    "engine": ins["engine"],
                                    "ins": [], "outs": [],
                                    "name": f"{ins['name']}-mw{k}",
                                    "opcode": "NoOp",
                                    "sync_info": {"on_wait": [wc],
                                                  "on_update": []},
                                })
                            si["on_wait"] = [w[-1]]
                    out.append(ins)
                blk["instructions"] = out

            def rec(o):
                if isinstance(o, dict):
                    if "instructions" in o:
                        fix_block(o)
                    for v in o.values():
                        rec(v)
                elif isinstance(o, list):
                    for v in o:
                        rec(v)

            rec(m)
            return _json.dumps(m).encode()

        _bass.Bass.to_json_bytes = _to_json_split
        _bass.Bass._mw_patched = True


def _build_module(with_b1: bool, with_b2: bool):
    import concourse.bass as bass
    import concourse.mybir as mybir
    from concourse.tile import TileContext

    _patch_tile_drain()

    FT = mybir.dt.float32r if USE_F32R else mybir.dt.float32
    F32 = mybir.dt.float32
    AFT = mybir.ActivationFunctionType

    BF16 = mybir.dt.bfloat16

    nc = bass.Bass()

    # ---- DRAM I/O ----
    # T0I: rows 0:64 = y0 shard FM, row 64 = ones (bias row)
    T0I_d = nc.dram_tensor("T0I", [D + 1, BS], FT, kind="ExternalInput")
    MW0_d = nc.dram_tensor("MW0", [D + 1, W], FT, kind="ExternalInput")
    MWK_d = nc.dram_tensor("MWK", [D, 15, W], FT, kind="ExternalInput")
    W1T_d = nc.dram_tensor("W1T", [128, 4, W], FT, kind="ExternalInput")
    W2TH_d = nc.dram_tensor("W2TH", [128, NINT, 4, D], FT, kind="ExternalInput")
    if with_b2:
        HB2_d = nc.dram_tensor("HB2", [1, NINT * D], FT, kind="ExternalInput")
    if with_b1:
        B1R_d = nc.dram_tensor("B1R", [1, W], FT, kind="ExternalInput")
    if with_b1 or with_b2:
        ONESR_d = nc.dram_tensor("ONESR", [1, BS], FT, kind="ExternalInput")
    UY_d = nc.dram_tensor("UY", [D + 1, D], FT, kind="ExternalInput")
    UK_d = nc.dram_tensor("UK", [D, 6 * D], FT, kind="ExternalInput")
    IDT_d = nc.dram_tensor("IDT", [D, D], FT, kind="ExternalInput")
    YS = nc.dram_tensor("YS", [NINT, D, BS], BF16, kind="ExternalOutput")

    with TileContext(nc) as tc:
        with (
            tc.tile_pool(name="const", bufs=1) as cpool,
            tc.tile_pool(name="state", bufs=1) as stpool,
            tc.tile_pool(name="work", bufs=3) as wpool,
            tc.tile_pool(name="zp", bufs=2, space="PSUM") as zpool,
            tc.tile_pool(name="hTp", bufs=2, space="PSUM") as hTpool,
            tc.tile_pool(name="kyp", bufs=2, space="PSUM") as kypool,
        ):
            # ---- constants -> SBUF ----
            MW0 = cpool.tile([D + 1, W], FT, tag="MW0")
            nc.sync.dma_start(MW0[:], MW0_d[:, :])
            MWK = cpool.tile([D, 15 * W], FT, tag="MWK")
            nc.sync.dma_start(MWK[:], MWK_d.rearrange("p k f -> p (k f)"))
            W1T = cpool.tile([128, 4 * W], FT, tag="W1T")
            nc.sync.dma_start(W1T[:], W1T_d.rearrange("p c f -> p (c f)"))
            W2TH = cpool.tile([128, NINT * 4 * D], FT, tag="W2TH")
            nc.sync.dma_start(W2TH[:], W2TH_d.rearrange("p i c f -> p (i c f)"))
            if with_b2:
                HB2 = cpool.tile([1, NINT * D], FT, tag="HB2")
                nc.sync.dma_start(HB2[:], HB2_d[:, :])
            if with_b1:
                B1R = cpool.tile([1, W], FT, tag="B1R")
                nc.sync.dma_start(B1R[:], B1R_d[:, :])
            UY = cpool.tile([D + 1, D], FT, tag="UY")
            nc.sync.dma_start(UY[:], UY_d[:, :])
            UK = cpool.tile([D, 6 * D], FT, tag="UK")
            nc.sync.dma_start(UK[:], UK_d[:, :])
            IDT = cpool.tile([D, D], FT, tag="IDT")
            nc.sync.dma_start(IDT[:], IDT_d[:, :])
            if with_b1 or with_b2:
                ONES = cpool.tile([1, BS], FT, tag="ONES")
                nc.sync.dma_start(ONES[:], ONESR_d[:, :])

            # ---- state ----
            # T0: rows 0:64 = y (FM), row 64 = ones (host-initialized DMA)
            T0 = stpool.tile([D + 1, BS], FT, tag="T0")
            nc.sync.dma_start(T0[:], T0I_d[:, :])
            K = [
                stpool.tile([D, BS], FT, tag=f"K{i}", name=f"K{i}")
                for i in range(6)
            ]

            mwk_idx = {}
            n = 0
            for j in range(2, 7):
                for i2 in range(len(A_ROWS[j])):
                    mwk_idx[(j, i2)] = n
                    n += 1

            def substep(i):
                for j in range(1, 7):
                    # ---- L0 (+ folded Tsit5 combination) -> z0 [64b, 512] BM
                    z0 = zpool.tile([BS, W], F32, tag="z")
                    terms = [(T0[:, :], MW0[:, :])]
                    for i2 in range(j - 1):
                        m = mwk_idx[(j, i2)]
                        terms.append((K[i2][:, :], MWK[:, m * W:(m + 1) * W]))
                    for c, (lhsT, rhs) in enumerate(terms):
                        nc.tensor.matmul(
                            z0[:], lhsT, rhs,
                            start=(c == 0), stop=(c == len(terms) - 1),
                        )
                    # ---- softplus -> h0 BM:
                    # r = relu(z-44); c = z-r (= min(z,44));
                    # out = ln(1+exp(c)) + r   (exact in fp32)
                    r0 = wpool.tile([BS, W], FT, tag="rp")
                    nc.vector.tensor_scalar(
                        r0[:], z0[:], 44.0, 0.0,
                        op0=mybir.AluOpType.subtract, op1=mybir.AluOpType.max,
                    )
                    c0 = wpool.tile([BS, W], FT, tag="cl")
                    nc.vector.tensor_sub(c0[:], z0[:], r0[:])
                    texp0 = wpool.tile([BS, W], FT, tag="texp")
                    nc.scalar.activation(texp0[:], c0[:], AFT.Exp)
                    s0 = wpool.tile([BS, W], FT, tag="sp")
                    nc.scalar.activation(s0[:], texp0[:], AFT.Ln, bias=1.0)
                    h0 = wpool.tile([BS, W], FT, tag="h")
                    nc.vector.tensor_add(h0[:], s0[:], r0[:])
                    # ---- transpose h0 -> FM [512, 64] as [128, 4*64]
                    h0Tp = hTpool.tile([128, 4 * BS], FT, tag="hTp")
                    for c in range(4):
                        nc.tensor.transpose(
                            h0Tp[:, c * BS:(c + 1) * BS],
                            h0[:, c * 128:(c + 1) * 128],
                            IDT[:],
                        )
                    h0T = wpool.tile([128, 4 * BS], FT, tag="hT")
                    nc.scalar.copy(h0T[:], h0Tp[:])
                    # ---- L1 -> z1 [64b, 512] BM (b1 via ones-row rank-1 mm)
                    z1 = zpool.tile([BS, W], F32, tag="z")
                    for c in range(4):
                        nc.tensor.matmul(
                            z1[:],
                            h0T[:, c * BS:(c + 1) * BS],
                            W1T[:, c * W:(c + 1) * W],
                            start=(c == 0), stop=(c == 3 and not with_b1),
                        )
                    if with_b1:
                        nc.tensor.matmul(
                            z1[:], ONES[:, :], B1R[:, :],
                            start=False, stop=True,
                        )
                    # ---- softplus -> h1 BM (same 4-op form)
                    r1 = wpool.tile([BS, W], FT, tag="rp")
                    nc.vector.tensor_scalar(
                        r1[:], z1[:], 44.0, 0.0,
                        op0=mybir.AluOpType.subtract, op1=mybir.AluOpType.max,
                    )
                    c1 = wpool.tile([BS, W], FT, tag="cl")
                    nc.vector.tensor_sub(c1[:], z1[:], r1[:])
                    texp1 = wpool.tile([BS, W], FT, tag="texp")
                    nc.scalar.activation(texp1[:], c1[:], AFT.Exp)
                    s1 = wpool.tile([BS, W], FT, tag="sp")
                    nc.scalar.activation(s1[:], texp1[:], AFT.Ln, bias=1.0)
                    h1 = wpool.tile([BS, W], FT, tag="h")
                    nc.vector.tensor_add(h1[:], s1[:], r1[:])
                    # ---- transpose h1 -> FM
                    h1Tp = hTpool.tile([128, 4 * BS], FT, tag="hTp")
                    for c in range(4):
                        nc.tensor.transpose(
                            h1Tp[:, c * BS:(c + 1) * BS],
                            h1[:, c * 128:(c + 1) * 128],
                            IDT[:],
                        )
                    h1T = wpool.tile([128, 4 * BS], FT, tag="hT")
                    nc.scalar.copy(h1T[:], h1Tp[:])
                    # ---- L2: khat_j = h*(W2 h1 + b2), FM [64d, 64b]
                    kp = kypool.tile([D, BS], F32, tag="k")
                    for c in range(4):
                        nc.tensor.matmul(
                            kp[:],
                            W2TH[:, (i * 4 + c) * D:(i * 4 + c + 1) * D],
                            h1T[:, c * BS:(c + 1) * BS],
                            start=(c == 0), stop=(c == 3 and not with_b2),
                        )
                    if with_b2:
                        nc.tensor.matmul(
                            kp[:],
                            HB2[:, i * D:(i + 1) * D],
                            ONES[:, :],
                            start=False, stop=True,
                        )
                    nc.vector.tensor_copy(K[j - 1][:], kp[:])

                # ---- y update: y += sum B_i khat_i
                yn = kypool.tile([D, BS], F32, tag="k")
                nc.tensor.matmul(yn[:], UY[:, :], T0[:, :], start=True, stop=False)
                for i2 in range(6):
                    nc.tensor.matmul(
                        yn[:],
                        UK[:, i2 * D:(i2 + 1) * D],
                        K[i2][:, :],
                        start=False, stop=(i2 == 5),
                    )
                nc.vector.tensor_copy(T0[0:D, :], yn[:])

            for i in range(NINT):
                for _s in range(SUBSTEPS):
                    substep(i)
                ybf = wpool.tile([D, BS], BF16, tag="ybf")
                nc.scalar.copy(ybf[:], T0[0:D, :])
                nc.sync.dma_start(YS[i, :, :], ybf[:])

    return nc


def _host_constants(ts, W0, b0, W1, b1, W2, b2):
    """Precompute all device constant tensors (fp32)."""
    f = np.float32
    ts = np.asarray(ts, f)
    W0, b0 = np.asarray(W0, f), np.asarray(b0, f)
    W1, b1 = np.asarray(W1, f), np.asarray(b1, f)
    W2, b2 = np.asarray(W2, f), np.asarray(b2, f)

    hs = (ts[1:] - ts[:-1]) / f(SUBSTEPS)          # [15]

    MW0 = np.zeros((D + 1, W), f)
    MW0[0:D, :] = W0.T                              # y rows
    MW0[D, :] = b0                                  # ones row -> +b0
    B1ROW = b1.reshape(1, W).copy()                 # [1, 512]

    MWK = np.zeros((D, 15, W), f)
    n = 0
    for j in range(2, 7):
        for a in A_ROWS[j]:
            MWK[:, n, :] = f(a) * W0.T
            n += 1

    W1T = np.zeros((128, 4, W), f)
    for c in range(4):
        W1T[:, c, :] = W1.T[c * 128:(c + 1) * 128, :]

    W2TH = np.zeros((128, NINT, 4, D), f)
    for i in range(NINT):
        for c in range(4):
            W2TH[:, i, c, :] = hs[i] * W2.T[c * 128:(c + 1) * 128, :]

    HB2 = np.zeros((1, NINT * D), f)
    for i in range(NINT):
        HB2[0, i * D:(i + 1) * D] = hs[i] * b2

    UY = np.zeros((D + 1, D), f)
    UY[0:D, 0:D] = np.eye(D, dtype=f)

    UK = np.zeros((D, 6 * D), f)
    for i2 in range(6):
        UK[:, i2 * D:(i2 + 1) * D] = f(B_W[i2]) * np.eye(D, dtype=f)

    IDT = np.eye(D, dtype=f)

    return dict(MW0=MW0, MWK=MWK, W1T=W1T, W2TH=W2TH, HB2=HB2,
                UY=UY, UK=UK, IDT=IDT, B1ROW=B1ROW)


def _split_consts(ts, W0, b0, W1, b1, W2, b2):
    consts = _host_constants(ts, W0, b0, W1, b1, W2, b2)
    b1row = consts.pop("B1ROW")
    with_b1 = bool(np.any(b1row != 0))
    with_b2 = bool(np.any(consts["HB2"] != 0))
    if with_b1:
        consts["B1R"] = b1row
    if not with_b2:
        consts.pop("HB2")
    if with_b1 or with_b2:
        consts["ONESR"] = np.ones((1, BS), np.float32)
    return consts, with_b1, with_b2


def _t0i_concat(y0):
    """Global [NCORES*(D+1), BS] T0 initializer (per-core shards stacked)."""
    t0i = np.empty((NCORES, D + 1, BS), np.float32)
    t0i[:, D, :] = 1.0
    for c in range(NCORES):
        t0i[c, 0:D, :] = y0[c * BS:(c + 1) * BS, :].T
    return t0i.reshape(NCORES * (D + 1), BS)


class _Runtime:
    """Cached compiled callable + device-resident constant inputs.

    run_bass_kernel_spmd rebuilds a fresh jax.jit closure (full XLA+NEFF
    recompile) and re-transfers every replicated constant on EVERY call.
    This caches both: the jit function is built once, constants are
    device_put once with the mesh sharding, and warm calls only upload the
    small y0-derived state tile and download YS.
    """

    def __init__(self, nc, consts):
        import jax
        import jax.numpy as jnp
        from jax.sharding import Mesh, PartitionSpec, NamedSharding
        try:
            from jax.experimental.shard_map import shard_map
        except ImportError:
            from jax import shard_map
        import concourse.mybir as mybir
        from concourse import bass2jax as b2j

        b2j.install_neuronx_cc_hook()

        self.jnp = jnp
        self.np = np

        in_names, out_names, out_avals = [], [], []
        partition_name = (
            nc.partition_id_tensor.name if nc.partition_id_tensor else None
        )
        for alloc in nc.m.functions[0].allocations:
            if not isinstance(alloc, mybir.MemoryLocationSet):
                continue
            name = alloc.memorylocations[0].name
            if alloc.kind == "ExternalInput":
                if name != partition_name:
                    in_names.append(name)
            elif alloc.kind == "ExternalOutput":
                out_names.append(name)
                out_avals.append(jax.core.ShapedArray(
                    tuple(alloc.tensor_shape), mybir.dt.np(alloc.dtype)))

        if nc.dbg_addr is not None and nc.dbg_callbacks:
            raise RuntimeError("dbg_callbacks unsupported in cached path")

        self.in_names = list(in_names)
        self.out_names = list(out_names)
        self.out_avals = list(out_avals)

        devices = jax.devices()[:NCORES]
        assert len(devices) == NCORES
        mesh = Mesh(np.asarray(devices), ("core",))
        P = PartitionSpec
        self.sharding = NamedSharding(mesh, P("core"))

        bind_in_names = list(in_names)
        if nc.dbg_addr is not None:
            bind_in_names.append(nc.dbg_addr.name)
        if partition_name is not None:
            bind_in_names.append(partition_name)
        n_args = len(bind_in_names) - (1 if partition_name is not None else 0)
        has_dbg = nc.dbg_addr is not None

        def _body(*args):
            operands = list(args)
            if partition_name is not None:
                operands.append(b2j.partition_id_tensor())
            outs = b2j._bass_exec_p.bind(
                *operands,
                out_avals=tuple(out_avals),
                in_names=tuple(bind_in_names),
                out_names=tuple(out_names),
                lowering_input_output_aliases=(),
                sim_require_finite=True,
                sim_require_nnan=True,
                nc=nc,
            )
            return tuple(outs)

        jitted = jax.jit(
            shard_map(
                _body, mesh=mesh,
                in_specs=(P("core"),) * n_args,
                out_specs=(P("core"),) * len(out_names),
                check_rep=False,
            ),
            keep_unused=True,
        )
        self.has_dbg = has_dbg

        # device-resident replicated constants (concat over cores, sharded)
        self.const_dev = {}
        for k, v in consts.items():
            g = np.concatenate([v] * NCORES, axis=0)
            self.const_dev[k] = jax.device_put(g, self.sharding)
        if has_dbg:
            self.const_dev["__dbg"] = jax.device_put(
                np.zeros((NCORES * 1, 2), np.uint32), self.sharding)

        # AOT-compile with the bass effect suppressed: C++ fast-path dispatch
        # on every warm call (no per-call Python retrace/dispatch overhead).
        in_shapes = []
        for name in self.in_names:
            if name == "T0I":
                in_shapes.append(jax.ShapeDtypeStruct(
                    (NCORES * (D + 1), BS), np.float32,
                    sharding=self.sharding))
            else:
                a = self.const_dev[name]
                in_shapes.append(jax.ShapeDtypeStruct(
                    a.shape, a.dtype, sharding=self.sharding))
        if has_dbg:
            a = self.const_dev["__dbg"]
            in_shapes.append(jax.ShapeDtypeStruct(
                a.shape, a.dtype, sharding=self.sharding))
        try:
            self.fn = b2j.fast_dispatch_compile(
                lambda: jitted.lower(*in_shapes).compile()
            )
        except Exception:
            self.fn = jitted

    def run(self, t0i_global):
        import jax
        args = []
        for name in self.in_names:
            if name == "T0I":
                args.append(jax.device_put(t0i_global, self.sharding))
            else:
                args.append(self.const_dev[name])
        if self.has_dbg:
            args.append(self.const_dev["__dbg"])
        outs = self.fn(*args)
        return {name: np.asarray(outs[i])
                for i, name in enumerate(self.out_names)}


def _kernel_fallback(consts, with_b1, with_b2, y0):
    from concourse.bass_utils import run_bass_kernel_spmd

    key = ("nc", with_b1, with_b2)
    if key not in _CACHE:
        _CACHE[key] = _build_module(with_b1, with_b2)
    nc = _CACHE[key]

    in_maps = []
    for c in range(NCORES):
        shard = y0[c * BS:(c + 1) * BS, :]          # [64, 64]
        t0i = np.zeros((D + 1, BS), np.float32)
        t0i[0:D, :] = shard.T
        t0i[D, :] = 1.0
        m = {"T0I": t0i}
        m.update({k: v for k, v in consts.items()})
        in_maps.append(m)

    res = run_bass_kernel_spmd(nc, in_maps, list(range(NCORES)))
    return [res.results[c]["YS"] for c in range(NCORES)]


def kernel(ts, y0, W0, b0, W1, b1, W2, b2):
    import sys

    consts, with_b1, with_b2 = _split_consts(ts, W0, b0, W1, b1, W2, b2)
    y0 = np.asarray(y0, np.float32)

    ys_per_core = None
    try:
        rt_key = ("rt", with_b1, with_b2)
        rt = _CACHE.get(rt_key)
        if rt is not None and not all(
            np.array_equal(rt[1][k], consts[k]) for k in consts
        ):
            rt = None                                # weights changed
        if rt is None:
            nc_key = ("nc", with_b1, with_b2)
            if nc_key not in _CACHE:
                _CACHE[nc_key] = _build_module(with_b1, with_b2)
            rt = (_Runtime(_CACHE[nc_key], consts), consts)
            _CACHE[rt_key] = rt
        res = rt[0].run(_t0i_concat(y0))
        ys = np.asarray(res["YS"]).astype(np.float32)
        ys = ys.reshape(NCORES, NINT, D, BS)
        ys_per_core = [ys[c] for c in range(NCORES)]
    except Exception as e:                           # pragma: no cover
        print(f"kernel: cached path failed ({e!r}); falling back",
              file=sys.stderr)
        ys_per_core = _kernel_fallback(consts, with_b1, with_b2, y0)

    out = np.zeros((B, T, D), np.float32)
    out[:, 0, :] = y0
    for c in range(NCORES):
        ys_c = np.asarray(ys_per_core[c]).astype(np.float32)
        out[c * BS:(c + 1) * BS, 1:, :] = ys_c.transpose(2, 0, 1)
    return out

